# revision 1
# baseline (speedup 1.0000x reference)
"""Trainium2 Bass kernel for nn_NeuralAttention (MLP-scored attention).

Math (per head h, batch 1, n=512, dh=64, P=32):
  qkv = x @ Wqkv^T, split 'b n (d k h) -> k b h n d'
  qp = q@Wq^T+bq ; kp = k@Wk^T+bk
  a  = qp@W1q^T  ; c = kp@W1k^T          (W1 = [W1q | W1k])
  h1 = relu(a_i + c_j + b1)              # [n, n, 32]
  h2 = relu(h1 @ W2^T + b2)              # [n, n, 16]
  s  = h2 @ W3^T (+ b3, drops in softmax)
  attn = softmax(causal(s)) ; out = attn @ v ; y = out @ Wout^T

Key algebra used:
  a = q @ (W1q Wq)^T + W1q bq  => Aq = W1q@Wq, fold consts into one
  per-partition constant s1const = W1q bq + W1k bk + b1.

Sharding: 16 heads over 8 cores (2 heads/core), Wout row-parallel;
host sums the 8 partial [1024, 512] outputs (unshard of row-parallel
layout) and transposes.

On-device layout ("j on partitions"): scores^T[j, i] computed in
j-tiles of 128; each 8-j "pair" only evaluates its causal i-suffix
(i >= j), ~37% less work than full rectangles.  Stage-1
relu(a_i + c_j) runs on DVE as tensor_scalar (bf16, 4x mode) with the
per-partition scalar = packed c columns; stage-2 is a block-diagonal
(4x) W2 matmul; stage-3 scatters 8-row score strips into 32-aligned
PSUM blocks via zero-padded W3 weights.  Each score tile is first
initialized by an identity-matmul with [causal -1e30 mask | zeros]
(start=True covers the full tile so PSUM has_written state stays
uniform - fresh-device garbage can be NaN); stage-3 then accumulates.
Softmax needs no max-subtraction (|s| << 1, and b3 cancels) and the
denominator comes free from a ones-column appended to V in the attn@V
matmul; normalization broadcasts 1/den via a K=1 ones-matmul.

Modeled (TimelineSim, calibrated cost model) per-core time: ~77.7 us;
PE busy ~57 us; measured rel. error vs fp32 reference: 3.9e-3.
"""

import sys

sys.path.insert(0, "/opt/trn_rl_repo")

from contextlib import ExitStack

import ml_dtypes
import numpy as np

import concourse.bass as bass
import concourse.tile as tile
from concourse import bacc, mybir
from concourse.bass_utils import run_bass_kernel_spmd

F32 = mybir.dt.float32
BF16 = mybir.dt.bfloat16
AF = mybir.ActivationFunctionType
ALU = mybir.AluOpType

B, N, DIM = 1, 512, 1024
HEADS, DH = 16, 64
P, P2 = 32, 16
N_CORES = 8
HPC = HEADS // N_CORES  # heads per core = 2

# scheduling tunables
TUNE = dict(
    s2_bufs=4,      # stage-2 psum pair tiles
    h1_bufs=24,     # stage-1 sbuf tiles
    h2_bufs=5,      # stage-2.5 sbuf tiles
    ex_bufs=3,      # exp sbuf tiles
    s25_dve_mod=4,  # every Nth stage-2.5 op goes to DVE (0 = none)
    s25_dve_mod_sm=2,  # same, for small tiles (t>=2)
    sc_bufs=1,      # scores/misc psum tiles (per head tag)
    op_bufs=1,      # out' accumulator psum tiles (per head tag)
    s1_pool_mod=0,  # every Nth stage-1 op goes to GPSIMD (0 = none)
)
NT = N // 128           # j tiles = 4
KT = DIM // 128         # contraction tiles for projections = 8


# ---------------------------------------------------------------- program ---

def build_program(repeat: int = 1):
    nc = bacc.Bacc("TRN2", target_bir_lowering=False, debug=False,
                   num_devices=N_CORES)

    d = {}
    def din(name, shape, dt):
        d[name] = nc.dram_tensor(name, shape, dt, kind="ExternalInput").ap()
        return d[name]

    xT_d = din("xT", [DIM, N], BF16)           # x transposed
    wqkT_d = din("wqkT", [DIM, 4 * DH], BF16)  # [q_h0 q_h1 k_h0 k_h1] lhsT
    wvT_d = din("wvT", [DIM, HPC * DH], BF16)  # v rhs (both heads)
    aqrep_d = din("aqrep", [128, 128], BF16)   # Aq^T replicated 4x, 2x rows
    akT_d = din("akT", [128, P], BF16)         # Ak^T stacked 2x rows
    s1c_d = din("s1c", [128, 1], F32)          # (W1q bq + W1k bk + b1) rep 4x
    w2b_d = din("w2b", [128, 64], BF16)        # blockdiag4(W2^T)
    b2r_d = din("b2r", [128, 1], F32)          # b2 tiled 8x
    w3s_d = din("w3s", [4, 128, P], BF16)      # scatter W3 blocks
    tri_d = din("tri", [128, N], BF16)         # [tri 0/-1e30 | zeros]
    id_d = din("iden", [128, 128], BF16)       # identity
    woutT_d = din("woutT", [HPC, DH, DIM], BF16)  # per-head Wout slice lhsT

    outT_d = nc.dram_tensor("outT", [DIM, N], F32, kind="ExternalOutput").ap()

    with tile.TileContext(nc) as tc, ExitStack() as ctx:
        cst = ctx.enter_context(tc.tile_pool(name="cst", bufs=1))

        # --- load constants / x (few big DMAs; k-tiles are column slices) ---
        wqk_big = cst.tile([128, KT * 4 * DH], BF16, tag="wqk")
        wqkv3 = wqkT_d.rearrange("(c a p) m -> p c a m", p=128, c=4)
        for cc in range(4):
            nc.sync.dma_start(
                wqk_big[:, cc * 2 * 4 * DH:(cc + 1) * 2 * 4 * DH], wqkv3[:, cc])
        x_big = cst.tile([128, KT * N], BF16, tag="xT16")
        xv3 = xT_d.rearrange("(c a p) n -> p c a n", p=128, c=4)
        for cc in range(4):
            nc.sync.dma_start(x_big[:, cc * 2 * N:(cc + 1) * 2 * N], xv3[:, cc])
        wv_big = cst.tile([128, KT * HPC * DH], BF16, tag="wv")
        nc.sync.dma_start(wv_big[:],
                          wvT_d.rearrange("(a p) m -> p a m", p=128))
        wqk = [wqk_big[:, kk * 4 * DH:(kk + 1) * 4 * DH] for kk in range(KT)]
        xT16 = [x_big[:, kk * N:(kk + 1) * N] for kk in range(KT)]
        wv = [wv_big[:, kk * HPC * DH:(kk + 1) * HPC * DH] for kk in range(KT)]
        warm = cst.tile([1, 4], F32, tag="warm")
        nc.vector.memset(warm[:], 0.0)
        nc.scalar.activation(warm[:], warm[:], AF.Exp)
        aqrep = cst.tile([128, 128], BF16, tag="aqrep")
        nc.sync.dma_start(aqrep[:], aqrep_d[:])
        akT = cst.tile([128, P], BF16, tag="akT")
        nc.sync.dma_start(akT[:], akT_d[:])
        s1c = cst.tile([128, 1], F32, tag="s1c")
        nc.sync.dma_start(s1c[:], s1c_d[:])
        w2b = cst.tile([128, 64], BF16, tag="w2b")
        nc.sync.dma_start(w2b[:], w2b_d[:])
        b2r = cst.tile([128, 1], F32, tag="b2r")
        nc.sync.dma_start(b2r[:], b2r_d[:])
        w3s = []
        for bb in range(4):
            t = cst.tile([128, P], BF16, tag=f"w3s_{bb}")
            nc.sync.dma_start(t[:], w3s_d[bb])
            w3s.append(t)
        tri = cst.tile([128, N], BF16, tag="tri")
        nc.sync.dma_start(tri[:], tri_d[:])
        iden = cst.tile([128, 128], BF16, tag="iden")
        nc.sync.dma_start(iden[:], id_d[:])
        woutT = []
        for h in range(HPC):
            t = cst.tile([DH, DIM], BF16, tag=f"woutT_{h}")
            nc.sync.dma_start(t[:], woutT_d[h])
            woutT.append(t)

        for rep in range(repeat):
            _body(nc, tc, ctx, rep, xT16, wqk, wv, aqrep, akT, s1c,
                  w2b, b2r, w3s, tri, iden, woutT, outT_d)

    nc.compile()
    return nc


def _body(nc, tc, ctx, rep, xT16, wqk, wv, aqrep, akT, s1c, w2b, b2r,
          w3s, tri, iden, woutT, outT_d):
    r = f"r{rep}"
    cst2 = ctx.enter_context(tc.tile_pool(name=f"cst2_{r}", bufs=1))

    # ------ P1: q/k projections -> q16/k16 [128, N] bf16 (2 heads stacked) --
    qk16 = []  # [q(2 heads), k(2 heads)]
    with tc.tile_pool(name=f"qkps_{r}", bufs=2, space="PSUM") as qkps:
        for m in range(2):
            ps = qkps.tile([128, N], F32, tag="qk")
            for kk in range(KT):
                nc.tensor.matmul(ps[:, :], wqk[kk][:, m * 128:(m + 1) * 128],
                                 xT16[kk][:, :],
                                 start=(kk == 0), stop=(kk == KT - 1))
            sb = cst2.tile([128, N], BF16, tag=f"qk16_{m}")
            nc.vector.tensor_copy(sb[:], ps[:])
            qk16.append(sb)

    # -------- P3: score MLP + softmax + attn@v, heads interleaved ----------
    out_h = []  # [64, N] bf16 normalized attention output per head
    with tc.tile_pool(name=f"s2_{r}", bufs=TUNE["s2_bufs"], space="PSUM") as s2ps, \
         tc.tile_pool(name=f"sc_{r}", bufs=TUNE["sc_bufs"], space="PSUM") as scps, \
         tc.tile_pool(name=f"op_{r}", bufs=TUNE["op_bufs"], space="PSUM") as ops, \
         tc.tile_pool(name=f"wk_{r}", bufs=TUNE["h1_bufs"]) as wk, \
         tc.tile_pool(name=f"h2_{r}", bufs=TUNE["h2_bufs"]) as h2p, \
         tc.tile_pool(name=f"ex_{r}", bufs=TUNE["ex_bufs"]) as exp_pool:

        a4s, cbs, op_pss = [], [], []
        for h in range(HPC):
            # a4 = 4x-replicated a^T (+ s1const via copy bias) [128, N] bf16
            a_ps = scps.tile([128, N], F32, tag=f"sc{h}")
            nc.tensor.matmul(a_ps[:, :], aqrep[64 * h:64 * (h + 1), :],
                             qk16[0][64 * h:64 * (h + 1), :],
                             start=True, stop=True, tile_position=(64 * h, 0))
            a4 = cst2.tile([128, N], BF16, tag=f"a4_{h}")
            nc.vector.tensor_scalar(a4[:], a_ps[:], s1c[:], None, ALU.add)
            a4s.append(a4)

            # cbias[32u+p, g] = (Ak k^T)[p, 4g+u]  [128, 128] f32
            c_ps = scps.tile([128, 128], F32, tag=f"sc{h}")
            k_re = qk16[1][64 * h:64 * (h + 1), :].rearrange(
                "d (g u) -> d u g", u=4)
            for u in range(4):
                nc.tensor.matmul(c_ps[32 * u:32 * (u + 1), :],
                                 akT[64 * h:64 * (h + 1), :],
                                 k_re[:, u, :], start=True, stop=True,
                                 tile_position=(64 * h, 32 * u))
            cb = cst2.tile([128, 128], F32, tag=f"cb_{h}")
            nc.vector.tensor_copy(cb[:], c_ps[:])
            cbs.append(cb)

            # out' accumulator [65, N] psum (num rows 0..64, den row 64)
            op_ps = ops.tile([65, N], F32, tag=f"op{h}")
            op_pss.append(op_ps)

        # ---- v projection -> v' [128, 130] bf16 per j-tile (deferred; uses
        # an s2 pool slot so it fills PE gaps during early scoring) ----
        vp = cst2.tile([128, NT * 130], BF16, tag="vp")
        for t in range(NT):
            ps_v = s2ps.tile([128, HPC * DH], F32, tag="s2")
            for kk in range(KT):
                nc.tensor.matmul(ps_v[:, :],
                                 xT16[kk][:, t * 128:(t + 1) * 128],
                                 wv[kk][:, :],
                                 start=(kk == 0), stop=(kk == KT - 1))
            for h in range(HPC):
                o0 = t * 130 + h * 65
                nc.scalar.copy(vp[:, o0:o0 + DH],
                               ps_v[:, h * DH:(h + 1) * DH])
                nc.vector.memset(vp[:, o0 + DH:o0 + 65], 1.0)

        for h in range(HPC):
            a4, cb, op_ps = a4s[h], cbs[h], op_pss[h]
            for t in range(NT):
                L = N - t * 128
                i0 = t * 128
                sc_ps = scps.tile([128, L], F32, tag=f"sc{h}")
                # init scores with causal mask (-1e30 in the i<j triangle,
                # 0 elsewhere); stage-3 matmuls then accumulate on top.
                nc.tensor.matmul(sc_ps[:, :], iden[:, :], tri[:, 0:L],
                                 start=True, stop=False,
                                 skip_group_check=True)
                # nm = pairs per stage-2 psum tile: small tiles merge pairs
                nm = 4 if L <= 128 else (2 if L <= 256 else 1)
                for m0 in range(0, 16, nm):
                    ofs = 8 * m0         # causal: pair m only needs i >= j
                    Lm = L - ofs
                    ps2 = s2ps.tile([128, nm * Lm], F32, tag="s2")
                    for dm in range(nm):
                        for v in range(2):
                            g = 32 * t + 2 * (m0 + dm) + v
                            h1 = wk.tile([128, Lm], BF16, tag="h1")
                            pm = TUNE["s1_pool_mod"]
                            eng = (nc.gpsimd if (pm and g % pm == pm - 1)
                                   else nc.vector)
                            eng.tensor_scalar(
                                h1[:], a4[:, i0 + ofs:N], cb[:, g:g + 1], 0.0,
                                ALU.add, ALU.max)
                            nc.tensor.matmul(
                                ps2[64 * v:64 * (v + 1),
                                    dm * Lm:(dm + 1) * Lm],
                                w2b[:, :], h1[:], start=True, stop=True)
                    h2 = h2p.tile([128, nm * Lm], BF16, tag="h2")
                    sm = TUNE["s25_dve_mod"]
                    if sm and (m0 // nm) % sm == sm - 1:
                        nc.vector.tensor_scalar(h2[:], ps2[:], b2r[:], 0.0,
                                                ALU.add, ALU.max)
                    else:
                        nc.scalar.activation(h2[:], ps2[:], AF.Relu,
                                             bias=b2r[:], scale=1.0)
                    for dm in range(nm):
                        m = m0 + dm
                        ab, bb = m // 4, m % 4
                        nc.tensor.matmul(
                            sc_ps[32 * ab:32 * (ab + 1), ofs:L],
                            w3s[bb][:, :], h2[:, dm * Lm:(dm + 1) * Lm],
                            start=False, stop=(m == 15),
                            skip_group_check=True,
                            tile_position=(0, 32 * ab))
                ex = exp_pool.tile([128, L], BF16, tag="ex")
                nc.scalar.activation(ex[:], sc_ps[:], AF.Exp)
                nc.tensor.matmul(op_ps[:, i0:N],
                                 vp[:, t * 130 + h * 65: t * 130 + h * 65 + 65],
                                 ex[:], start=(t == 0), stop=(t == NT - 1),
                                 skip_group_check=True)

        for h in range(HPC):
            # normalize: out = num * (1/den)
            op_ps = op_pss[h]
            num = cst2.tile([DH, N], F32, tag=f"num_{h}")
            nc.scalar.copy(num[:], op_ps[0:DH, :])
            rsb = cst2.tile([128, N], F32, tag=f"rec_{h}")
            nc.vector.reciprocal(rsb[64:65, :], op_ps[64:65, :])
            ones = cst2.tile([128, DH], F32, tag=f"ones_{h}")
            nc.vector.memset(ones[64:65, :], 1.0)
            rb_ps = scps.tile([DH, N], F32, tag=f"sc{h}")
            nc.tensor.matmul(rb_ps[:, :], ones[64:65, :], rsb[64:65, :],
                             start=True, stop=True)
            o = cst2.tile([DH, N], BF16, tag=f"out_{h}")
            nc.vector.tensor_mul(o[:], num[:], rb_ps[:])
            out_h.append(o)

    # ---------------- P4: output projection (row-parallel Wout) ------------
    with tc.tile_pool(name=f"wo_{r}", bufs=4, space="PSUM") as wops, \
         tc.tile_pool(name=f"ob_{r}", bufs=4) as obp:
        for ot in range(KT):
            ps = wops.tile([128, N], F32, tag="wo")
            for h in range(HPC):
                nc.tensor.matmul(ps[:, :],
                                 woutT[h][:, ot * 128:(ot + 1) * 128],
                                 out_h[h][:, :],
                                 start=(h == 0), stop=(h == HPC - 1))
            ob = obp.tile([128, N], F32, tag="ob")
            if ot % 2 == 0:
                nc.vector.tensor_copy(ob[:], ps[:])
            else:
                nc.scalar.copy(ob[:], ps[:])
            nc.sync.dma_start(
                outT_d.rearrange("(a p) n -> a p n", p=128)[ot], ob[:])


# ---------------------------------------------------------------- host side -

def prep_inputs(x, Wqkv, Wout, Wq, bq, Wk, bk, W1, b1, W2, b2, W3, b3):
    """Build the per-core input maps (all numpy, fp32/bf16)."""
    x = np.asarray(x, np.float32).reshape(N, DIM)
    Wqkv = np.asarray(Wqkv, np.float32)
    Wout = np.asarray(Wout, np.float32)
    Wq, bq = np.asarray(Wq, np.float32), np.asarray(bq, np.float32)
    Wk, bk = np.asarray(Wk, np.float32), np.asarray(bk, np.float32)
    W1, b1 = np.asarray(W1, np.float32), np.asarray(b1, np.float32)
    W2, b2 = np.asarray(W2, np.float32), np.asarray(b2, np.float32)
    W3 = np.asarray(W3, np.float32)

    bf = lambda a: np.ascontiguousarray(a).astype(ml_dtypes.bfloat16)
    f32 = lambda a: np.ascontiguousarray(a, np.float32)

    xT = bf(x.T)                                    # [DIM, N]

    W1q, W1k = W1[:, :P], W1[:, P:]
    Aq = W1q @ Wq                                   # [32, 64]
    Ak = W1k @ Wk
    s1const = W1q @ bq + W1k @ bk + b1              # [32]
    s1c = f32(np.tile(s1const, 4)[:, None])         # [128, 1]

    aqrep = np.zeros((128, 128), np.float32)
    for u in range(4):
        aqrep[0:DH, 32 * u:32 * (u + 1)] = Aq.T
    aqrep[DH:128] = aqrep[0:DH]
    akT = np.concatenate([Ak.T, Ak.T], axis=0)      # [128, 32]

    w2b = np.zeros((128, 64), np.float32)
    for u in range(4):
        w2b[32 * u:32 * (u + 1), 16 * u:16 * (u + 1)] = W2.T
    b2r = f32(np.tile(b2, 8)[:, None])              # [128, 1]

    w3s = np.zeros((4, 128, P), np.float32)
    for bb in range(4):
        for v in range(2):
            for u in range(4):
                col = 8 * bb + 4 * v + u
                for q in range(P2):
                    w3s[bb, 64 * v + 16 * u + q, col] = W3[0, q]

    ii = np.arange(128)
    tri = np.zeros((128, N), np.float32)        # [j, i]: 0 valid, -1e30 not
    tri[:, 0:128] = np.where(ii[None, :] >= ii[:, None], 0.0, -1e30)
    iden = np.eye(128, dtype=np.float32)

    # per-head channel index in Wqkv output: o = d*48 + k*16 + h
    dch = np.arange(DH)
    in_maps = []
    for c in range(N_CORES):
        h0, h1 = HPC * c, HPC * c + 1
        rows_q = [dch * 48 + 0 * HEADS + h for h in (h0, h1)]
        rows_k = [dch * 48 + 1 * HEADS + h for h in (h0, h1)]
        rows_v = [dch * 48 + 2 * HEADS + h for h in (h0, h1)]
        wqkT = np.concatenate(
            [Wqkv[r] for r in rows_q + rows_k], axis=0).T     # [DIM, 256]
        wvT = np.concatenate([Wqkv[r] for r in rows_v], axis=0).T  # [DIM,128]
        woutT = np.stack(
            [Wout[:, DH * h:DH * (h + 1)].T for h in (h0, h1)])  # [2,64,DIM]
        in_maps.append({
            "xT": xT,
            "wqkT": bf(wqkT),
            "wvT": bf(wvT),
            "aqrep": bf(aqrep),
            "akT": bf(akT),
            "s1c": s1c,
            "w2b": bf(w2b),
            "b2r": b2r,
            "w3s": bf(w3s),
            "tri": bf(tri),
            "iden": bf(iden),
            "woutT": bf(woutT),
        })
    return in_maps


_PROGRAM_CACHE = {}


def _get_program(repeat=1):
    if repeat not in _PROGRAM_CACHE:
        _PROGRAM_CACHE[repeat] = build_program(repeat)
    return _PROGRAM_CACHE[repeat]


def run(in_maps, repeat=1):
    nc = _get_program(repeat)
    return run_bass_kernel_spmd(nc, in_maps, list(range(N_CORES)))


def kernel(**inputs) -> np.ndarray:
    in_maps = prep_inputs(**inputs)
    res = run(in_maps)
    acc = np.zeros((DIM, N), np.float64)
    for c in range(N_CORES):
        acc += res.results[c]["outT"].astype(np.float64)
    return np.ascontiguousarray(acc.T.astype(np.float32)).reshape(B, N, DIM)



# revision 12
# speedup vs baseline: 1.1002x; 1.1002x over previous
"""Trainium2 Bass kernel for nn_NeuralAttention (MLP-scored attention).

Math (per head h, batch 1, n=512, dh=64, P=32):
  qkv = x @ Wqkv^T, split 'b n (d k h) -> k b h n d'
  qp = q@Wq^T+bq ; kp = k@Wk^T+bk
  a  = qp@W1q^T  ; c = kp@W1k^T          (W1 = [W1q | W1k])
  h1 = relu(a_i + c_j + b1)              # [n, n, 32]
  h2 = relu(h1 @ W2^T + b2)              # [n, n, 16]
  s  = h2 @ W3^T (+ b3, drops in softmax)
  attn = softmax(causal(s)) ; out = attn @ v ; y = out @ Wout^T

Key algebra: Aq = W1q@Wq, Ak = W1k@Wk, s1const = W1q bq + W1k bk + b1.

Sharding: 16 heads over 8 cores (2 heads/core), Wout row-parallel; host
sums the 8 partial [1024, 512] outputs (fp16) and transposes.

fp8 usage (validated vs fp32 reference, rel err ~3e-3):
  - q/k projection: x*SX and Wqkv*SW in fp8e4, DoubleRow matmuls
    (K=256/instr, 0.5 cyc/col); scale F=SX*SW folded out of Aq/Ak.
  - stage-2 weights W2 scaled by S4 so psum z = S4*(h1@W2^T); stage-2.5
    emits h2s = relu(z + S4 b2) = S4*h2 in fp8e4.
  - stage-3: DoubleRow fp8 matmuls, scatter weights w3f = W3*(SC/S4)
    full-height [128, 2, 128] per (ab, parity) so out base partition
    stays 0 (DR codegen requires it); two j-pairs (16 j) per matmul.
  - exp applies scale 1/SC; mask (-1e30, bf16 identity-matmul init)
    stays enormous after scaling so masked lanes exp to 0.

On-device layout ("j on partitions"): scores^T[j, i] in j-tiles of 128;
pair m (8 j) evaluates i >= 8*(m&~1) (causal, quantized to DR pairs).
Stage-1 relu(a_i + c_j) is tensor_scalar (bf16, DVE 4x) spread over
DVE/GPSIMD/Act; stage-2 is the 4x block-diagonal W2 matmul (bf16);
stage-2.5 relu+bias emits fp8 on Act/DVE. Softmax denominator comes
from a ones-column in the attn@v matmul; normalization multiplies by
the broadcast reciprocal straight out of PSUM.

P4 packs both heads (K=128) into 8 matmuls, psum copied to fp16 and
DMA'd in 2 big transfers. Input DMAs are consolidated into 8 loads.
"""

import sys

sys.path.insert(0, "/opt/trn_rl_repo")

from contextlib import ExitStack

import ml_dtypes
import numpy as np

import concourse.bass as bass
import concourse.tile as tile
from concourse import bacc, mybir
from concourse.bass_utils import run_bass_kernel_spmd

F32 = mybir.dt.float32
F16 = mybir.dt.float16
BF16 = mybir.dt.bfloat16
F8 = mybir.dt.float8e4
AF = mybir.ActivationFunctionType
ALU = mybir.AluOpType
DR = mybir.MatmulPerfMode.DoubleRow

B, N, DIM = 1, 512, 1024
HEADS, DH = 16, 64
P, P2 = 32, 16
N_CORES = 8
HPC = HEADS // N_CORES  # heads per core = 2
NT = N // 128            # j tiles = 4
KT = DIM // 128          # contraction tiles for projections = 8

SX, SW = 16.0, 512.0     # fp8 scales: x, Wqkv (qk slice)
S4 = 512.0               # h2 scale (folded into w2b/b2r)
SC = 2048.0              # score scale; exp uses 1/SC

# scheduling tunables
TUNE = dict(
    s2_bufs=4,       # stage-2 psum tiles
    h1_bufs=24,      # stage-1 sbuf tiles
    h2_bufs=5,       # h2 sbuf tiles
    ex_bufs=3,       # exp sbuf tiles
    s1_pool_mod=5,   # every Nth stage-1 op -> GPSIMD (0 = none)
    s1_act_mod=0,    # every Nth stage-1 op -> Act (0 = none)
    s25_dve_mod=3,   # every Nth stage-2.5 op -> DVE (0 = none)
)


# ---------------------------------------------------------------- program ---

def build_program(repeat: int = 1):
    nc = bacc.Bacc("TRN2", target_bir_lowering=False, debug=False,
                   num_devices=N_CORES)

    d = {}
    def din(name, shape, dt):
        d[name] = nc.dram_tensor(name, shape, dt, kind="ExternalInput").ap()
        return d[name]

    x8_d = din("x8", [128, KT * N], F8)          # x*SX transposed, fp8
    wqk8_d = din("wqk8", [128, KT * 4 * DH], F8)  # [q_h0 q_h1 k_h0 k_h1]*SW
    xT_d = din("xT", [DIM, N], BF16)             # x transposed (v proj)
    wvT_d = din("wvT", [DIM, HPC * DH], BF16)    # v rhs (both heads)
    cstB_d = din("cstB", [128, 864], BF16)       # aqrep|akT|w2b|iden|tri
    cstF_d = din("cstF", [128, 2], F32)          # s1c | b2r
    w3f_d = din("w3f", [128, 8 * 256], F8)       # (ab,pi) scatter weights
    wo2_d = din("wo2", [128, DIM], BF16)         # packed Wout slice lhsT

    outT_d = nc.dram_tensor("outT", [DIM, N], F16, kind="ExternalOutput").ap()

    with tile.TileContext(nc) as tc, ExitStack() as ctx:
        cst = ctx.enter_context(tc.tile_pool(name="cst", bufs=1))

        # --- consolidated input DMAs (order matters: qk path first) ---
        x8 = cst.tile([128, KT * N], F8, tag="x8")
        nc.sync.dma_start(x8[:], x8_d[:])
        wqk8 = cst.tile([128, KT * 4 * DH], F8, tag="wqk8")
        nc.sync.dma_start(wqk8[:], wqk8_d[:])
        cstF = cst.tile([128, 2], F32, tag="cstF")
        nc.sync.dma_start(cstF[:], cstF_d[:])
        cstB = cst.tile([128, 864], BF16, tag="cstB")
        nc.sync.dma_start(cstB[:], cstB_d[:])
        w3f_big = cst.tile([128, 8 * 256], F8, tag="w3f")
        nc.sync.dma_start(w3f_big[:], w3f_d[:])
        x_big = cst.tile([128, KT * N], BF16, tag="xT16")
        nc.sync.dma_start(x_big[:], xT_d.rearrange("(a p) n -> p a n", p=128))
        wv_big = cst.tile([128, KT * HPC * DH], BF16, tag="wv")
        nc.sync.dma_start(wv_big[:], wvT_d.rearrange("(a p) m -> p a m", p=128))
        woutT = []
        for h in range(HPC):
            t = cst.tile([DH, DIM], BF16, tag=f"woutT_{h}")
            nc.sync.dma_start(t[:], wo2_d[DH * h:DH * (h + 1), :])
            woutT.append(t)

        aqrep = cstB[:, 0:128]
        akT = cstB[:, 128:160]
        w2b = cstB[:, 160:224]
        iden = cstB[:, 224:352]
        tri = cstB[:, 352:864]
        s1c = cstF[:, 0:1]
        b2r = cstF[:, 1:2]
        w3f = [w3f_big[:, k * 256:(k + 1) * 256] for k in range(8)]
        xT16 = [x_big[:, kk * N:(kk + 1) * N] for kk in range(KT)]
        wv = [wv_big[:, kk * HPC * DH:(kk + 1) * HPC * DH] for kk in range(KT)]

        # exp table warm-up
        warm = cst.tile([1, 4], F32, tag="warm")
        nc.vector.memset(warm[:], 0.0)
        nc.scalar.activation(warm[:], warm[:], AF.Exp)

        for rep in range(repeat):
            _body(nc, tc, ctx, rep, x8, wqk8, xT16, wv, aqrep, akT, s1c,
                  w2b, b2r, w3f, tri, iden, woutT, outT_d)

    nc.compile()
    return nc


def _body(nc, tc, ctx, rep, x8, wqk8, xT16, wv, aqrep, akT, s1c, w2b, b2r,
          w3f, tri, iden, woutT, outT_d):
    r = f"r{rep}"
    cst2 = ctx.enter_context(tc.tile_pool(name=f"cst2_{r}", bufs=1))

    # ------ P1: q/k projections (fp8 DoubleRow) -> qk16 [128, N] bf16 ------
    qk16 = []  # [q(2 heads), k(2 heads)]
    with tc.tile_pool(name=f"qkps_{r}", bufs=2, space="PSUM") as qkps:
        for m in range(2):
            ps = qkps.tile([128, N], F32, tag="qk")
            for kp in range(KT // 2):
                lhs = wqk8[:, kp * 512:(kp + 1) * 512] \
                    .rearrange("p (two c) -> p two c", two=2) \
                    [:, :, m * 128:(m + 1) * 128]
                rhs = x8[:, kp * 2 * N:(kp + 1) * 2 * N] \
                    .rearrange("p (two n) -> p two n", two=2)
                nc.tensor.matmul(ps[:, :], lhs, rhs,
                                 start=(kp == 0), stop=(kp == KT // 2 - 1),
                                 perf_mode=DR)
            sb = cst2.tile([128, N], BF16, tag=f"qk16_{m}")
            nc.vector.tensor_copy(sb[:], ps[:])
            qk16.append(sb)

    # -------- P3: score MLP + softmax + attn@v, heads interleaved ----------
    out_h = []  # [64, N] bf16 normalized attention output per head
    with tc.tile_pool(name=f"s2_{r}", bufs=TUNE["s2_bufs"], space="PSUM") as s2ps, \
         tc.tile_pool(name=f"sc_{r}", bufs=1, space="PSUM") as scps, \
         tc.tile_pool(name=f"op_{r}", bufs=1, space="PSUM") as ops, \
         tc.tile_pool(name=f"wk_{r}", bufs=TUNE["h1_bufs"]) as wk, \
         tc.tile_pool(name=f"h2_{r}", bufs=TUNE["h2_bufs"]) as h2p, \
         tc.tile_pool(name=f"ex_{r}", bufs=TUNE["ex_bufs"]) as exp_pool:

        a4s, cbs, op_pss = [], [], []
        for h in range(HPC):
            # a4 = 4x-replicated a^T (+ s1const via scalar add) [128, N] bf16
            a_ps = scps.tile([128, N], F32, tag=f"sc{h}")
            nc.tensor.matmul(a_ps[:, :], aqrep[64 * h:64 * (h + 1), :],
                             qk16[0][64 * h:64 * (h + 1), :],
                             start=True, stop=True, tile_position=(64 * h, 0))
            a4 = cst2.tile([128, N], BF16, tag=f"a4_{h}")
            nc.vector.tensor_scalar(a4[:], a_ps[:], s1c[:], None, ALU.add)
            a4s.append(a4)

            # cbias[32u+p, g] = (Ak k^T)[p, 4g+u]  [128, 128] f32
            c_ps = scps.tile([128, 128], F32, tag=f"sc{h}")
            k_re = qk16[1][64 * h:64 * (h + 1), :].rearrange(
                "d (g u) -> d u g", u=4)
            for u in range(4):
                nc.tensor.matmul(c_ps[32 * u:32 * (u + 1), :],
                                 akT[64 * h:64 * (h + 1), :],
                                 k_re[:, u, :], start=True, stop=True,
                                 tile_position=(64 * h, 32 * u))
            cb = cst2.tile([128, 128], F32, tag=f"cb_{h}")
            nc.vector.tensor_copy(cb[:], c_ps[:])
            cbs.append(cb)

            # out' accumulator [65, N] psum (num rows 0..64, den row 64)
            op_ps = ops.tile([65, N], F32, tag=f"op{h}")
            op_pss.append(op_ps)

        # ---- v projection -> v' [128, 130] bf16 per j-tile (deferred) ----
        vp = cst2.tile([128, NT * 130], BF16, tag="vp")
        for t in range(NT):
            ps_v = s2ps.tile([128, HPC * DH], F32, tag="s2")
            for kk in range(KT):
                nc.tensor.matmul(ps_v[:, :],
                                 xT16[kk][:, t * 128:(t + 1) * 128],
                                 wv[kk][:, :],
                                 start=(kk == 0), stop=(kk == KT - 1))
            for h in range(HPC):
                o0 = t * 130 + h * 65
                nc.scalar.copy(vp[:, o0:o0 + DH],
                               ps_v[:, h * DH:(h + 1) * DH])
                nc.vector.memset(vp[:, o0 + DH:o0 + 65], 1.0)

        s1_n = [0]  # stage-1 round-robin counter
        s25_n = [0]

        def s1_engine():
            s1_n[0] += 1
            pm, am = TUNE["s1_pool_mod"], TUNE["s1_act_mod"]
            if pm and s1_n[0] % pm == 0:
                return "pool"
            if am and s1_n[0] % am == am - 1:
                return "act"
            return "dve"

        def emit_s1(h1, a4, i0ofs, cb, g):
            eng = s1_engine()
            if eng == "act":
                nc.scalar.activation(h1, a4[:, i0ofs:N], AF.Relu,
                                     bias=cb[:, g:g + 1], scale=1.0)
            else:
                e = nc.gpsimd if eng == "pool" else nc.vector
                e.tensor_scalar(h1, a4[:, i0ofs:N], cb[:, g:g + 1], 0.0,
                                ALU.add, ALU.max)

        def emit_s25(h2ap, psap):
            s25_n[0] += 1
            sm = TUNE["s25_dve_mod"]
            if sm and s25_n[0] % sm == 0:
                nc.vector.tensor_scalar(h2ap, psap, b2r[:], 0.0,
                                        ALU.add, ALU.max)
            else:
                nc.scalar.activation(h2ap, psap, AF.Relu, bias=b2r[:],
                                     scale=1.0)

        for h in range(HPC):
            a4, cb, op_ps = a4s[h], cbs[h], op_pss[h]
            for t in range(NT):
                L = N - t * 128
                i0 = t * 128
                sc_ps = scps.tile([128, L], F32, tag=f"sc{h}")
                # causal mask init (-1e30 above diagonal); stage-3 accumulates.
                nc.tensor.matmul(sc_ps[:, :], iden[:, :], tri[:, 0:L],
                                 start=True, stop=False,
                                 skip_group_check=True)
                # nm = pairs per stage-2 psum tile (2 = DR pairing unit);
                # psum tile must stay within one 2KB bank (<=512 f32 cols).
                nm = 2 if L <= 256 else 1
                for m0 in range(0, 16, 2):
                    ofs = 8 * m0
                    Lm = L - ofs
                    h2t = h2p.tile([128, 2 * Lm], F8, tag="h2")
                    for half in range(2):       # pair m0+half
                        m = m0 + half
                        if nm == 1:
                            ps2 = s2ps.tile([128, Lm], F32, tag="s2")
                            for v in range(2):
                                g = 32 * t + 2 * m + v
                                h1 = wk.tile([128, Lm], BF16, tag="h1")
                                emit_s1(h1[:], a4, i0 + ofs, cb, g)
                                nc.tensor.matmul(
                                    ps2[64 * v:64 * (v + 1), :],
                                    w2b[:, :], h1[:], start=True, stop=True)
                            emit_s25(h2t[:, half * Lm:(half + 1) * Lm],
                                     ps2[:])
                        elif half == 0:
                            ps2 = s2ps.tile([128, 2 * Lm], F32, tag="s2")
                            for dm in range(2):
                                for v in range(2):
                                    g = 32 * t + 2 * (m0 + dm) + v
                                    h1 = wk.tile([128, Lm], BF16, tag="h1")
                                    emit_s1(h1[:], a4, i0 + ofs, cb, g)
                                    nc.tensor.matmul(
                                        ps2[64 * v:64 * (v + 1),
                                            dm * Lm:(dm + 1) * Lm],
                                        w2b[:, :], h1[:],
                                        start=True, stop=True)
                            emit_s25(h2t[:], ps2[:])
                    # stage-3: one DoubleRow matmul for pair (m0, m0+1)
                    ab, pi = m0 // 4, (m0 // 2) % 2
                    nc.tensor.matmul(
                        sc_ps[:, ofs:L],
                        w3f[2 * ab + pi].rearrange(
                            "p (two c) -> p two c", two=2),
                        h2t[:].rearrange("p (two n) -> p two n", two=2),
                        start=False, stop=(m0 + 2 >= 16),
                        skip_group_check=True, perf_mode=DR)
                ex = exp_pool.tile([128, L], BF16, tag="ex")
                nc.scalar.activation(ex[:], sc_ps[:], AF.Exp, scale=1.0 / SC)
                nc.tensor.matmul(op_ps[:, i0:N],
                                 vp[:, t * 130 + h * 65: t * 130 + h * 65 + 65],
                                 ex[:], start=(t == 0), stop=(t == NT - 1),
                                 skip_group_check=True)

        # normalize: out = num * (1/den) per head
        for h in range(HPC):
            op_ps = op_pss[h]
            rsb = cst2.tile([128, N], F32, tag=f"rec_{h}")
            nc.vector.reciprocal(rsb[64:65, :], op_ps[64:65, :])
            ones = cst2.tile([128, DH], F32, tag=f"ones_{h}")
            nc.vector.memset(ones[64:65, :], 1.0)
            rb_ps = scps.tile([DH, N], F32, tag=f"sc{h}")
            nc.tensor.matmul(rb_ps[:, :], ones[64:65, :], rsb[64:65, :],
                             start=True, stop=True)
            rb16 = cst2.tile([DH, N], BF16, tag=f"rb16_{h}")
            nc.scalar.copy(rb16[:], rb_ps[:])
            o = cst2.tile([DH, N], BF16, tag=f"out_{h}")
            nc.vector.tensor_mul(o[:], op_ps[0:DH, :], rb16[:])
            out_h.append(o)

    # ---------------- P4: output projection (row-parallel Wout) ------------
    with tc.tile_pool(name=f"wo_{r}", bufs=4, space="PSUM") as wops, \
         tc.tile_pool(name=f"ob_{r}", bufs=2) as obp:
        ob = None
        for ot in range(KT):
            ps = wops.tile([128, N], F32, tag="wo")
            for h in range(HPC):
                nc.tensor.matmul(ps[:, :],
                                 woutT[h][:, ot * 128:(ot + 1) * 128],
                                 out_h[h][:, :],
                                 start=(h == 0), stop=(h == HPC - 1))
            if ot % 4 == 0:
                ob = obp.tile([128, 4 * N], F16, tag="ob")
            if ot % 2 == 0:
                nc.vector.tensor_copy(ob[:, (ot % 4) * N:(ot % 4 + 1) * N],
                                      ps[:])
            else:
                nc.scalar.copy(ob[:, (ot % 4) * N:(ot % 4 + 1) * N], ps[:])
            if ot % 4 == 3:
                nc.sync.dma_start(
                    outT_d.rearrange("(c a p) n -> p c a n", p=128, c=2)
                    [:, ot // 4],
                    ob[:].rearrange("p (a n) -> p a n", a=4))


# ---------------------------------------------------------------- host side -

def prep_inputs(x, Wqkv, Wout, Wq, bq, Wk, bk, W1, b1, W2, b2, W3, b3):
    """Build the per-core input maps (all numpy)."""
    x = np.asarray(x, np.float32).reshape(N, DIM)
    Wqkv = np.asarray(Wqkv, np.float32)
    Wout = np.asarray(Wout, np.float32)
    Wq, bq = np.asarray(Wq, np.float32), np.asarray(bq, np.float32)
    Wk, bk = np.asarray(Wk, np.float32), np.asarray(bk, np.float32)
    W1, b1 = np.asarray(W1, np.float32), np.asarray(b1, np.float32)
    W2, b2 = np.asarray(W2, np.float32), np.asarray(b2, np.float32)
    W3 = np.asarray(W3, np.float32)

    bf = lambda a: np.ascontiguousarray(a).astype(ml_dtypes.bfloat16)
    f8 = lambda a: np.ascontiguousarray(a).astype(ml_dtypes.float8_e4m3)
    f32 = lambda a: np.ascontiguousarray(a, np.float32)

    xT = x.T                                        # [DIM, N]
    # x8 fp8 layout [128, KT*N]: col kk*N+n, row p -> x[n, kk*128+p]*SX
    x8 = f8(xT.reshape(KT, 128, N).transpose(1, 0, 2).reshape(128, KT * N)
            * SX)

    F = SX * SW
    W1q, W1k = W1[:, :P], W1[:, P:]
    Aq = (W1q @ Wq) / F                             # descale fp8 qk psum
    Ak = (W1k @ Wk) / F
    s1const = W1q @ bq + W1k @ bk + b1              # [32]

    aqrep = np.zeros((128, 128), np.float32)
    for u in range(4):
        aqrep[0:DH, 32 * u:32 * (u + 1)] = Aq.T
    aqrep[DH:128] = aqrep[0:DH]
    akT = np.concatenate([Ak.T, Ak.T], axis=0)      # [128, 32]

    w2b = np.zeros((128, 64), np.float32)           # blockdiag4(S4*W2^T)
    for u in range(4):
        w2b[32 * u:32 * (u + 1), 16 * u:16 * (u + 1)] = W2.T * S4

    ii = np.arange(128)
    tri = np.zeros((128, N), np.float32)        # [j, i]: 0 valid, -1e30 not
    tri[:, 0:128] = np.where(ii[None, :] >= ii[:, None], 0.0, -1e30)
    iden = np.eye(128, dtype=np.float32)

    cstB = np.concatenate(
        [aqrep, akT, w2b, iden, tri], axis=1)       # [128, 864]
    cstF = np.stack([np.tile(s1const, 4), np.tile(b2 * S4, 8)],
                    axis=1)                         # [128, 2] f32

    # full-height DR scatter weights: w3f[2*ab+pi][p, i, col] nonzero at
    # col = 32*ab + 8*bb + 4*v + u for bb = 2*pi + i, p = 64v+16u+q.
    w3v = W3[0] * (SC / S4)                         # [16]
    w3full = np.zeros((8, 128, 2, 128), np.float32)
    for ab in range(4):
        for pi in range(2):
            for i in range(2):
                bb = 2 * pi + i
                for v in range(2):
                    for u in range(4):
                        col = 32 * ab + 8 * bb + 4 * v + u
                        for q in range(P2):
                            w3full[2 * ab + pi, 64 * v + 16 * u + q, i,
                                   col] = w3v[q]

    # per-head channel index in Wqkv output: o = d*48 + k*16 + h
    dch = np.arange(DH)
    in_maps = []
    for c in range(N_CORES):
        h0, h1h = HPC * c, HPC * c + 1
        rows_q = [dch * 48 + 0 * HEADS + h for h in (h0, h1h)]
        rows_k = [dch * 48 + 1 * HEADS + h for h in (h0, h1h)]
        rows_v = [dch * 48 + 2 * HEADS + h for h in (h0, h1h)]
        wqkT = np.concatenate(
            [Wqkv[r] for r in rows_q + rows_k], axis=0).T     # [DIM, 256]
        # wqk8 fp8 layout [128, KT*4DH]
        wqk8 = f8(wqkT.reshape(KT, 128, 4 * DH).transpose(1, 0, 2)
                  .reshape(128, KT * 4 * DH) * SW)
        wvT = np.concatenate([Wqkv[r] for r in rows_v], axis=0).T  # [DIM,128]
        wo2 = np.concatenate(
            [Wout[:, DH * h:DH * (h + 1)].T for h in (h0, h1h)])  # [128,DIM]
        in_maps.append({
            "x8": x8,
            "wqk8": wqk8,
            "xT": bf(xT),
            "wvT": bf(wvT),
            "cstB": bf(cstB),
            "cstF": f32(cstF),
            "w3f": f8(w3full.transpose(1, 0, 2, 3).reshape(128, 8 * 256)),
            "wo2": bf(wo2),
        })
    return in_maps


_PROGRAM_CACHE = {}


def _get_program(repeat=1):
    if repeat not in _PROGRAM_CACHE:
        _PROGRAM_CACHE[repeat] = build_program(repeat)
    return _PROGRAM_CACHE[repeat]


def run(in_maps, repeat=1):
    nc = _get_program(repeat)
    return run_bass_kernel_spmd(nc, in_maps, list(range(N_CORES)))


def kernel(**inputs) -> np.ndarray:
    in_maps = prep_inputs(**inputs)
    res = run(in_maps)
    acc = np.zeros((DIM, N), np.float64)
    for c in range(N_CORES):
        acc += res.results[c]["outT"].astype(np.float64)
    return np.ascontiguousarray(acc.T.astype(np.float32)).reshape(B, N, DIM)


# revision 13
# speedup vs baseline: 1.1342x; 1.0308x over previous
"""Trainium2 Bass kernel for nn_NeuralAttention (MLP-scored attention).

Math (per head h, batch 1, n=512, dh=64, P=32):
  qkv = x @ Wqkv^T, split 'b n (d k h) -> k b h n d'
  qp = q@Wq^T+bq ; kp = k@Wk^T+bk
  a  = qp@W1q^T  ; c = kp@W1k^T          (W1 = [W1q | W1k])
  h1 = relu(a_i + c_j + b1)              # [n, n, 32]
  h2 = relu(h1 @ W2^T + b2)              # [n, n, 16]
  s  = h2 @ W3^T (+ b3, drops in softmax)
  attn = softmax(causal(s)) ; out = attn @ v ; y = out @ Wout^T

Key algebra: Aq = W1q@Wq, Ak = W1k@Wk, s1const = W1q bq + W1k bk + b1.

Sharding: 16 heads over 8 cores (2 heads/core), Wout row-parallel; host
sums the 8 partial [1024, 512] outputs (fp16) and transposes.

fp8 usage (validated vs fp32 reference, rel err ~3e-3):
  - q/k projection: x*SX and Wqkv*SW in fp8e4, DoubleRow matmuls
    (K=256/instr, 0.5 cyc/col); scale F=SX*SW folded out of Aq/Ak.
  - stage-2 weights W2 scaled by S4 so psum z = S4*(h1@W2^T); stage-2.5
    emits h2s = relu(z + S4 b2) = S4*h2 in fp8e4.
  - stage-3: DoubleRow fp8 matmuls, scatter weights w3f = W3*(SC/S4)
    full-height [128, 2, 128] per (ab, parity) so out base partition
    stays 0 (DR codegen requires it); two j-pairs (16 j) per matmul.
  - exp applies scale 1/SC; mask (-1e30, bf16 identity-matmul init)
    stays enormous after scaling so masked lanes exp to 0.

On-device layout ("j on partitions"): scores^T[j, i] in j-tiles of 128;
pair m (8 j) evaluates i >= 8*(m&~1) (causal, quantized to DR pairs).
Stage-1 relu(a_i + c_j) is tensor_scalar (bf16, DVE 4x) spread over
DVE/GPSIMD/Act; stage-2 is the 4x block-diagonal W2 matmul (bf16);
stage-2.5 relu+bias emits fp8 on Act/DVE. Softmax denominator comes
from a ones-column in the attn@v matmul; normalization multiplies by
the broadcast reciprocal straight out of PSUM.

P4 packs both heads (K=128) into 8 matmuls, psum copied to fp16 and
DMA'd in 2 big transfers. Input DMAs are consolidated into 8 loads.
"""

import sys

sys.path.insert(0, "/opt/trn_rl_repo")

from contextlib import ExitStack

import ml_dtypes
import numpy as np

import concourse.bass as bass
import concourse.tile as tile
from concourse import bacc, mybir
from concourse.bass_utils import run_bass_kernel_spmd

F32 = mybir.dt.float32
F16 = mybir.dt.float16
BF16 = mybir.dt.bfloat16
F8 = mybir.dt.float8e4
AF = mybir.ActivationFunctionType
ALU = mybir.AluOpType
DR = mybir.MatmulPerfMode.DoubleRow

B, N, DIM = 1, 512, 1024
HEADS, DH = 16, 64
P, P2 = 32, 16
N_CORES = 8
HPC = HEADS // N_CORES  # heads per core = 2
NT = N // 128            # j tiles = 4
KT = DIM // 128          # contraction tiles for projections = 8

SX, SW = 16.0, 512.0     # fp8 scales: x, Wqkv (qk slice)
S4 = 512.0               # h2 scale (folded into w2b/b2r)
SC = 2048.0              # score scale; exp uses 1/SC

# scheduling tunables
TUNE = dict(
    s2_bufs=4,       # stage-2 psum tiles
    h1_bufs=24,      # stage-1 sbuf tiles
    h2_bufs=5,       # h2 sbuf tiles
    ex_bufs=3,       # exp sbuf tiles
    s1_pool_mod=3,   # every Nth stage-1 op -> GPSIMD (0 = none)
    s1_act_mod=0,    # every Nth stage-1 op -> Act (0 = none)
    s25_dve_mod=4,   # every Nth stage-2.5 op -> DVE (0 = none)
)


# ---------------------------------------------------------------- program ---

def build_program(repeat: int = 1):
    nc = bacc.Bacc("TRN2", target_bir_lowering=False, debug=False,
                   num_devices=N_CORES)

    d = {}
    def din(name, shape, dt):
        d[name] = nc.dram_tensor(name, shape, dt, kind="ExternalInput").ap()
        return d[name]

    x8_d = din("x8", [128, KT * N], F8)          # x*SX transposed, fp8
    wqk8_d = din("wqk8", [128, KT * 4 * DH], F8)  # [q_h0 q_h1 k_h0 k_h1]*SW
    xT_d = din("xT", [DIM, N], BF16)             # x transposed (v proj)
    wvT_d = din("wvT", [DIM, HPC * DH], BF16)    # v rhs (both heads)
    cstB_d = din("cstB", [128, 864], BF16)       # aqrep|akT|w2b|iden|tri
    cstF_d = din("cstF", [128, 2], F32)          # s1c | b2r
    w3f_d = din("w3f", [128, 8 * 256], F8)       # (ab,pi) scatter weights
    wo2_d = din("wo2", [128, DIM], BF16)         # packed Wout slice lhsT

    outT_d = nc.dram_tensor("outT", [DIM, N], F16, kind="ExternalOutput").ap()

    with tile.TileContext(nc) as tc, ExitStack() as ctx:
        cst = ctx.enter_context(tc.tile_pool(name="cst", bufs=1))

        # --- consolidated input DMAs (order matters: qk path first) ---
        x8 = cst.tile([128, KT * N], F8, tag="x8")
        wqk8 = cst.tile([128, KT * 4 * DH], F8, tag="wqk8")
        nc.sync.dma_start(wqk8[:], wqk8_d[:])
        nc.sync.dma_start(x8[:, 0:KT * N // 2], x8_d[:, 0:KT * N // 2])
        nc.sync.dma_start(x8[:, KT * N // 2:], x8_d[:, KT * N // 2:])
        cstF = cst.tile([128, 2], F32, tag="cstF")
        nc.sync.dma_start(cstF[:], cstF_d[:])
        cstB = cst.tile([128, 864], BF16, tag="cstB")
        nc.sync.dma_start(cstB[:], cstB_d[:])
        w3f_big = cst.tile([128, 8 * 256], F8, tag="w3f")
        nc.sync.dma_start(w3f_big[:], w3f_d[:])
        x_big = cst.tile([128, KT * N], BF16, tag="xT16")
        nc.sync.dma_start(x_big[:], xT_d.rearrange("(a p) n -> p a n", p=128))
        wv_big = cst.tile([128, KT * HPC * DH], BF16, tag="wv")
        nc.sync.dma_start(wv_big[:], wvT_d.rearrange("(a p) m -> p a m", p=128))
        woutT = []
        for h in range(HPC):
            t = cst.tile([DH, DIM], BF16, tag=f"woutT_{h}")
            nc.sync.dma_start(t[:], wo2_d[DH * h:DH * (h + 1), :])
            woutT.append(t)

        aqrep = cstB[:, 0:128]
        akT = cstB[:, 128:160]
        w2b = cstB[:, 160:224]
        iden = cstB[:, 224:352]
        tri = cstB[:, 352:864]
        s1c = cstF[:, 0:1]
        b2r = cstF[:, 1:2]
        w3f = [w3f_big[:, k * 256:(k + 1) * 256] for k in range(8)]
        xT16 = [x_big[:, kk * N:(kk + 1) * N] for kk in range(KT)]
        wv = [wv_big[:, kk * HPC * DH:(kk + 1) * HPC * DH] for kk in range(KT)]

        # exp table warm-up
        warm = cst.tile([1, 4], F32, tag="warm")
        nc.vector.memset(warm[:], 0.0)
        nc.scalar.activation(warm[:], warm[:], AF.Exp)

        for rep in range(repeat):
            _body(nc, tc, ctx, rep, x8, wqk8, xT16, wv, aqrep, akT, s1c,
                  w2b, b2r, w3f, tri, iden, woutT, outT_d)

    nc.compile()
    return nc


def _body(nc, tc, ctx, rep, x8, wqk8, xT16, wv, aqrep, akT, s1c, w2b, b2r,
          w3f, tri, iden, woutT, outT_d):
    r = f"r{rep}"
    cst2 = ctx.enter_context(tc.tile_pool(name=f"cst2_{r}", bufs=1))

    # ------ P1: q/k projections (fp8 DoubleRow) -> qk16 [128, N] bf16 ------
    qk16 = []  # [q(2 heads), k(2 heads)]
    with tc.tile_pool(name=f"qkps_{r}", bufs=2, space="PSUM") as qkps:
        for m in range(2):
            ps = qkps.tile([128, N], F32, tag="qk")
            for kp in range(KT // 2):
                lhs = wqk8[:, kp * 512:(kp + 1) * 512] \
                    .rearrange("p (two c) -> p two c", two=2) \
                    [:, :, m * 128:(m + 1) * 128]
                rhs = x8[:, kp * 2 * N:(kp + 1) * 2 * N] \
                    .rearrange("p (two n) -> p two n", two=2)
                nc.tensor.matmul(ps[:, :], lhs, rhs,
                                 start=(kp == 0), stop=(kp == KT // 2 - 1),
                                 perf_mode=DR)
            sb = cst2.tile([128, N], BF16, tag=f"qk16_{m}")
            nc.vector.tensor_copy(sb[:], ps[:])
            qk16.append(sb)

    # -------- P3: score MLP + softmax + attn@v, heads interleaved ----------
    out_h = []  # [64, N] bf16 normalized attention output per head
    with tc.tile_pool(name=f"s2_{r}", bufs=TUNE["s2_bufs"], space="PSUM") as s2ps, \
         tc.tile_pool(name=f"sc_{r}", bufs=1, space="PSUM") as scps, \
         tc.tile_pool(name=f"op_{r}", bufs=1, space="PSUM") as ops, \
         tc.tile_pool(name=f"wk_{r}", bufs=TUNE["h1_bufs"]) as wk, \
         tc.tile_pool(name=f"h2_{r}", bufs=TUNE["h2_bufs"]) as h2p, \
         tc.tile_pool(name=f"ex_{r}", bufs=TUNE["ex_bufs"]) as exp_pool:

        a4s, cbs, op_pss = [], [], []
        for h in range(HPC):
            # a4 = 4x-replicated a^T (+ s1const via scalar add) [128, N] bf16
            a_ps = scps.tile([128, N], F32, tag=f"sc{h}")
            nc.tensor.matmul(a_ps[:, :], aqrep[64 * h:64 * (h + 1), :],
                             qk16[0][64 * h:64 * (h + 1), :],
                             start=True, stop=True, tile_position=(64 * h, 0))
            a4 = cst2.tile([128, N], BF16, tag=f"a4_{h}")
            nc.vector.tensor_scalar(a4[:], a_ps[:], s1c[:], None, ALU.add)
            a4s.append(a4)

            # cbias[32u+p, g] = (Ak k^T)[p, 4g+u]  [128, 128] f32
            c_ps = scps.tile([128, 128], F32, tag=f"sc{h}")
            k_re = qk16[1][64 * h:64 * (h + 1), :].rearrange(
                "d (g u) -> d u g", u=4)
            for u in range(4):
                nc.tensor.matmul(c_ps[32 * u:32 * (u + 1), :],
                                 akT[64 * h:64 * (h + 1), :],
                                 k_re[:, u, :], start=True, stop=True,
                                 tile_position=(64 * h, 32 * u))
            cb = cst2.tile([128, 128], F32, tag=f"cb_{h}")
            nc.vector.tensor_copy(cb[:], c_ps[:])
            cbs.append(cb)

            # out' accumulator [65, N] psum (num rows 0..64, den row 64)
            op_ps = ops.tile([65, N], F32, tag=f"op{h}")
            op_pss.append(op_ps)

        # ---- v projection -> v' [128, 130] bf16 per j-tile (deferred) ----
        vp = cst2.tile([128, NT * 130], BF16, tag="vp")
        for t in range(NT):
            ps_v = s2ps.tile([128, HPC * DH], F32, tag="s2")
            for kk in range(KT):
                nc.tensor.matmul(ps_v[:, :],
                                 xT16[kk][:, t * 128:(t + 1) * 128],
                                 wv[kk][:, :],
                                 start=(kk == 0), stop=(kk == KT - 1))
            for h in range(HPC):
                o0 = t * 130 + h * 65
                nc.scalar.copy(vp[:, o0:o0 + DH],
                               ps_v[:, h * DH:(h + 1) * DH])
                nc.vector.memset(vp[:, o0 + DH:o0 + 65], 1.0)

        s1_n = [0]  # stage-1 round-robin counter
        s25_n = [0]

        def s1_engine():
            s1_n[0] += 1
            pm, am = TUNE["s1_pool_mod"], TUNE["s1_act_mod"]
            if pm and s1_n[0] % pm == 0:
                return "pool"
            if am and s1_n[0] % am == am - 1:
                return "act"
            return "dve"

        def emit_s1(h1, a4, i0ofs, cb, g):
            eng = s1_engine()
            if eng == "act":
                nc.scalar.activation(h1, a4[:, i0ofs:N], AF.Relu,
                                     bias=cb[:, g:g + 1], scale=1.0)
            else:
                e = nc.gpsimd if eng == "pool" else nc.vector
                e.tensor_scalar(h1, a4[:, i0ofs:N], cb[:, g:g + 1], 0.0,
                                ALU.add, ALU.max)

        def emit_s25(h2ap, psap):
            s25_n[0] += 1
            sm = TUNE["s25_dve_mod"]
            if sm and s25_n[0] % sm == 0:
                nc.vector.tensor_scalar(h2ap, psap, b2r[:], 0.0,
                                        ALU.add, ALU.max)
            else:
                nc.scalar.activation(h2ap, psap, AF.Relu, bias=b2r[:],
                                     scale=1.0)

        for t in range(NT):
            for h in range(HPC):
                a4, cb, op_ps = a4s[h], cbs[h], op_pss[h]
                L = N - t * 128
                i0 = t * 128
                sc_ps = scps.tile([128, L], F32, tag=f"sc{h}")
                # causal mask init (-1e30 above diagonal); stage-3 accumulates.
                nc.tensor.matmul(sc_ps[:, :], iden[:, :], tri[:, 0:L],
                                 start=True, stop=False,
                                 skip_group_check=True)
                # nm = pairs per stage-2 psum tile (2 = DR pairing unit);
                # psum tile must stay within one 2KB bank (<=512 f32 cols).
                nm = 2 if L <= 256 else 1
                for m0 in range(0, 16, 2):
                    ofs = 8 * m0
                    Lm = L - ofs
                    h2t = h2p.tile([128, 2 * Lm], F8, tag="h2")
                    for half in range(2):       # pair m0+half
                        m = m0 + half
                        if nm == 1:
                            ps2 = s2ps.tile([128, Lm], F32, tag="s2")
                            for v in range(2):
                                g = 32 * t + 2 * m + v
                                h1 = wk.tile([128, Lm], BF16, tag="h1")
                                emit_s1(h1[:], a4, i0 + ofs, cb, g)
                                nc.tensor.matmul(
                                    ps2[64 * v:64 * (v + 1), :],
                                    w2b[:, :], h1[:], start=True, stop=True)
                            emit_s25(h2t[:, half * Lm:(half + 1) * Lm],
                                     ps2[:])
                        elif half == 0:
                            ps2 = s2ps.tile([128, 2 * Lm], F32, tag="s2")
                            for dm in range(2):
                                for v in range(2):
                                    g = 32 * t + 2 * (m0 + dm) + v
                                    h1 = wk.tile([128, Lm], BF16, tag="h1")
                                    emit_s1(h1[:], a4, i0 + ofs, cb, g)
                                    nc.tensor.matmul(
                                        ps2[64 * v:64 * (v + 1),
                                            dm * Lm:(dm + 1) * Lm],
                                        w2b[:, :], h1[:],
                                        start=True, stop=True)
                            emit_s25(h2t[:], ps2[:])
                    # stage-3: one DoubleRow matmul for pair (m0, m0+1)
                    ab, pi = m0 // 4, (m0 // 2) % 2
                    nc.tensor.matmul(
                        sc_ps[:, ofs:L],
                        w3f[2 * ab + pi].rearrange(
                            "p (two c) -> p two c", two=2),
                        h2t[:].rearrange("p (two n) -> p two n", two=2),
                        start=False, stop=(m0 + 2 >= 16),
                        skip_group_check=True, perf_mode=DR)
                ex = exp_pool.tile([128, L], BF16, tag="ex")
                nc.scalar.activation(ex[:], sc_ps[:], AF.Exp, scale=1.0 / SC)
                nc.tensor.matmul(op_ps[:, i0:N],
                                 vp[:, t * 130 + h * 65: t * 130 + h * 65 + 65],
                                 ex[:], start=(t == 0), stop=(t == NT - 1),
                                 skip_group_check=True)
                if t == NT - 1:
                    # normalize this head: out = num * (1/den)
                    rsb = cst2.tile([128, N], F32, tag=f"rec_{h}")
                    nc.vector.reciprocal(rsb[64:65, :], op_ps[64:65, :])
                    ones = cst2.tile([128, DH], F32, tag=f"ones_{h}")
                    nc.vector.memset(ones[64:65, :], 1.0)
                    rb_ps = scps.tile([DH, N], F32, tag=f"sc{h}")
                    nc.tensor.matmul(rb_ps[:, :], ones[64:65, :],
                                     rsb[64:65, :], start=True, stop=True)
                    rb16 = cst2.tile([DH, N], BF16, tag=f"rb16_{h}")
                    nc.scalar.copy(rb16[:], rb_ps[:])
                    o = cst2.tile([DH, N], BF16, tag=f"out_{h}")
                    nc.vector.tensor_mul(o[:], op_ps[0:DH, :], rb16[:])
                    out_h.append(o)

    # ---------------- P4: output projection (row-parallel Wout) ------------
    with tc.tile_pool(name=f"wo_{r}", bufs=4, space="PSUM") as wops, \
         tc.tile_pool(name=f"ob_{r}", bufs=2) as obp:
        ob = None
        for ot in range(KT):
            ps = wops.tile([128, N], F32, tag="wo")
            for h in range(HPC):
                nc.tensor.matmul(ps[:, :],
                                 woutT[h][:, ot * 128:(ot + 1) * 128],
                                 out_h[h][:, :],
                                 start=(h == 0), stop=(h == HPC - 1))
            if ot % 2 == 0:
                ob = obp.tile([128, 2 * N], F16, tag="ob")
                nc.vector.tensor_copy(ob[:, 0:N], ps[:])
            else:
                nc.scalar.copy(ob[:, N:2 * N], ps[:])
                nc.sync.dma_start(
                    outT_d.rearrange("(c a p) n -> p c a n", p=128, c=4)
                    [:, ot // 2],
                    ob[:].rearrange("p (a n) -> p a n", a=2))


# ---------------------------------------------------------------- host side -

def prep_inputs(x, Wqkv, Wout, Wq, bq, Wk, bk, W1, b1, W2, b2, W3, b3):
    """Build the per-core input maps (all numpy)."""
    x = np.asarray(x, np.float32).reshape(N, DIM)
    Wqkv = np.asarray(Wqkv, np.float32)
    Wout = np.asarray(Wout, np.float32)
    Wq, bq = np.asarray(Wq, np.float32), np.asarray(bq, np.float32)
    Wk, bk = np.asarray(Wk, np.float32), np.asarray(bk, np.float32)
    W1, b1 = np.asarray(W1, np.float32), np.asarray(b1, np.float32)
    W2, b2 = np.asarray(W2, np.float32), np.asarray(b2, np.float32)
    W3 = np.asarray(W3, np.float32)

    bf = lambda a: np.ascontiguousarray(a).astype(ml_dtypes.bfloat16)
    f8 = lambda a: np.ascontiguousarray(a).astype(ml_dtypes.float8_e4m3)
    f32 = lambda a: np.ascontiguousarray(a, np.float32)

    xT = x.T                                        # [DIM, N]
    # x8 fp8 layout [128, KT*N]: col kk*N+n, row p -> x[n, kk*128+p]*SX
    x8 = f8(xT.reshape(KT, 128, N).transpose(1, 0, 2).reshape(128, KT * N)
            * SX)

    F = SX * SW
    W1q, W1k = W1[:, :P], W1[:, P:]
    Aq = (W1q @ Wq) / F                             # descale fp8 qk psum
    Ak = (W1k @ Wk) / F
    s1const = W1q @ bq + W1k @ bk + b1              # [32]

    aqrep = np.zeros((128, 128), np.float32)
    for u in range(4):
        aqrep[0:DH, 32 * u:32 * (u + 1)] = Aq.T
    aqrep[DH:128] = aqrep[0:DH]
    akT = np.concatenate([Ak.T, Ak.T], axis=0)      # [128, 32]

    w2b = np.zeros((128, 64), np.float32)           # blockdiag4(S4*W2^T)
    for u in range(4):
        w2b[32 * u:32 * (u + 1), 16 * u:16 * (u + 1)] = W2.T * S4

    ii = np.arange(128)
    tri = np.zeros((128, N), np.float32)        # [j, i]: 0 valid, -1e30 not
    tri[:, 0:128] = np.where(ii[None, :] >= ii[:, None], 0.0, -1e30)
    iden = np.eye(128, dtype=np.float32)

    cstB = np.concatenate(
        [aqrep, akT, w2b, iden, tri], axis=1)       # [128, 864]
    cstF = np.stack([np.tile(s1const, 4), np.tile(b2 * S4, 8)],
                    axis=1)                         # [128, 2] f32

    # full-height DR scatter weights: w3f[2*ab+pi][p, i, col] nonzero at
    # col = 32*ab + 8*bb + 4*v + u for bb = 2*pi + i, p = 64v+16u+q.
    w3v = W3[0] * (SC / S4)                         # [16]
    w3full = np.zeros((8, 128, 2, 128), np.float32)
    for ab in range(4):
        for pi in range(2):
            for i in range(2):
                bb = 2 * pi + i
                for v in range(2):
                    for u in range(4):
                        col = 32 * ab + 8 * bb + 4 * v + u
                        for q in range(P2):
                            w3full[2 * ab + pi, 64 * v + 16 * u + q, i,
                                   col] = w3v[q]

    # per-head channel index in Wqkv output: o = d*48 + k*16 + h
    dch = np.arange(DH)
    in_maps = []
    for c in range(N_CORES):
        h0, h1h = HPC * c, HPC * c + 1
        rows_q = [dch * 48 + 0 * HEADS + h for h in (h0, h1h)]
        rows_k = [dch * 48 + 1 * HEADS + h for h in (h0, h1h)]
        rows_v = [dch * 48 + 2 * HEADS + h for h in (h0, h1h)]
        wqkT = np.concatenate(
            [Wqkv[r] for r in rows_q + rows_k], axis=0).T     # [DIM, 256]
        # wqk8 fp8 layout [128, KT*4DH]
        wqk8 = f8(wqkT.reshape(KT, 128, 4 * DH).transpose(1, 0, 2)
                  .reshape(128, KT * 4 * DH) * SW)
        wvT = np.concatenate([Wqkv[r] for r in rows_v], axis=0).T  # [DIM,128]
        wo2 = np.concatenate(
            [Wout[:, DH * h:DH * (h + 1)].T for h in (h0, h1h)])  # [128,DIM]
        in_maps.append({
            "x8": x8,
            "wqk8": wqk8,
            "xT": bf(xT),
            "wvT": bf(wvT),
            "cstB": bf(cstB),
            "cstF": f32(cstF),
            "w3f": f8(w3full.transpose(1, 0, 2, 3).reshape(128, 8 * 256)),
            "wo2": bf(wo2),
        })
    return in_maps


_PROGRAM_CACHE = {}


def _get_program(repeat=1):
    if repeat not in _PROGRAM_CACHE:
        _PROGRAM_CACHE[repeat] = build_program(repeat)
    return _PROGRAM_CACHE[repeat]


def run(in_maps, repeat=1):
    nc = _get_program(repeat)
    return run_bass_kernel_spmd(nc, in_maps, list(range(N_CORES)))


def kernel(**inputs) -> np.ndarray:
    in_maps = prep_inputs(**inputs)
    res = run(in_maps)
    acc = np.zeros((DIM, N), np.float64)
    for c in range(N_CORES):
        acc += res.results[c]["outT"].astype(np.float64)
    return np.ascontiguousarray(acc.T.astype(np.float32)).reshape(B, N, DIM)
